# revision 2
# baseline (speedup 1.0000x reference)
"""Multi-head attention TRN2 Bass kernel (8 NeuronCores, SPMD).

Problem: B=4, S=1024, E=1024, H=16 heads of dim 64, fp32.
    Q = q @ Wq^T (per head), K, V likewise
    scores = Q K^T / 8 ; P = softmax(scores) ; ctx = P V
    out = concat_heads(ctx) @ Wo^T

Sharding: core c handles batch b = c // 2 and head-group g = c % 2
(8 heads each). Each core computes a partial output projection over its
512 concat features; the host sums the two partials per batch.

Schedule (v2): the kernel is one software pipeline ordered around two
near-co-critical engines: PE (~82us of bf16 matmul streaming) and the
scalar engine (~73us of exp). Key structure:
  - 8 dummy warm-up matmuls on a zeroed scratch tile run during the
    framework preamble so the PE HAM clock-gate is released (2.4 GHz)
    by the time real matmuls start.
  - All input DMAs are issued on ONE queue (sync) in strict priority
    order, chunked so compute chases the chunks: wq/wk pair-0 blocks,
    xq, xk (these gate the first exp), then the remaining w pairs,
    then wv/xv (V proj), then wo. Pair-major wq/wk HBM layout lets
    pair 0 land first.
  - Attention is emitted sh-major: (0,0),(0,1) as scores+exp only
    (ctx deferred; the 16 eAB tiles buffer in SBUF), then vproj once
    xv lands, then the deferred ctx+normalize, then blocks (0,2..3),
    (1,0..3) inline. The exp stream on the scalar engine starts at
    ~18us and ideally never starves after.
  - Output projection: wave0 (out rows 0:511, needs only sh=0 cat
    columns) runs as 8 one-bank groups slotted 2-per-block under the
    sh=1 attention blocks (PE has slack there while ACT catches up);
    wave1 (rows 512:1023) runs at the end on the freed score banks,
    with pair-3 contributions deferred so only 8 matmuls + drains
    trail the last normalize.
  - Output is stored bf16 (halves the tail DMA); host sums in fp32.
  - PSUM: pp_sc 2x[128,1024] scores, pp_ctx 2x[128,512] ctx/denom,
    pp_mm 2x[128,512] projections + wave0 = 8 banks exactly.
  - softmax without max-subtraction (scores ~N(0,1): exp is safe);
    normalization = reciprocal_approx_fast on the denominator rows
    (emitted by the augmented-V ones blocks) + one DVE multiply; the
    cross-partition denominator broadcast rides the gpsimd DMA queue
    so it never queues behind input loads.
"""

from contextlib import ExitStack

import ml_dtypes
import numpy as np

import concourse.bacc as bacc
import concourse.mybir as mybir
import concourse.tile as tile
from concourse.bass_utils import run_bass_kernel_spmd

B, S, E, H = 4, 1024, 1024, 16
HD = 64          # head dim
HPC = 8          # heads per core
NPAIR = 4        # head pairs per core
NET = 8          # e-tiles (E / 128)
NTT = 8          # t-tiles (S / 128)
P = 128

F32 = mybir.dt.float32
BF16 = mybir.dt.bfloat16
EXP = mybir.ActivationFunctionType.Exp
SCALE = 1.0 / 8.0  # 1/sqrt(HD)
BF = ml_dtypes.bfloat16


def _emit(nc, tc, ctx, aps):
    xqT, xkT, xvT, wqT, wkT, wvT, woT, out = aps

    const = ctx.enter_context(tc.tile_pool(name="const", bufs=1))
    etp = ctx.enter_context(tc.tile_pool(name="etp", bufs=16))
    obp = ctx.enter_context(tc.tile_pool(name="obp", bufs=3))
    rcp = ctx.enter_context(tc.tile_pool(name="rcp", bufs=8))
    pp_mm = ctx.enter_context(tc.tile_pool(name="pp_mm", bufs=2, space="PSUM"))
    pp_sc = ctx.enter_context(tc.tile_pool(name="pp_sc", bufs=2, space="PSUM"))
    pp_ctx = ctx.enter_context(tc.tile_pool(name="pp_ctx", bufs=2, space="PSUM"))

    wo_t = const.tile([P, 4096], BF16, name="wo_t")
    qt = const.tile([P, 4096], BF16, name="qt")
    kt = const.tile([P, 4096], BF16, name="kt")
    vaug = const.tile([P, 8192], BF16, name="vaug")
    cat = const.tile([P, 4096], BF16, name="cat")
    xq = const.tile([P, 8192], BF16, name="xq")
    xk = const.tile([P, 8192], BF16, name="xk")
    xv = const.tile([P, 8192], BF16, name="xv")
    wq = const.tile([P, 4096], BF16, name="wq")
    wk = const.tile([P, 4096], BF16, name="wk")
    wv = const.tile([P, 4096], BF16, name="wv")
    scr = const.tile([P, 512], BF16, name="scr")

    # scratch for PE warm-up + ones blocks of the V augmentation
    nc.gpsimd.memset(scr[:], 0.0)
    v4 = vaug[:, :].rearrange("p (j q c) -> p j q c", q=2, c=P)
    nc.gpsimd.memset(v4[:, :, 0, HD:P], 1.0)
    nc.gpsimd.memset(v4[:, :, 1, 0:HD], 1.0)

    # ---- PE warm-up: 8 dummy matmuls release the HAM clock gate while
    # the first input DMAs are still in flight ----
    psd = pp_mm.tile([P, 512], F32, name="psd", tag="mm")
    for i in range(8):
        nc.tensor.matmul(psd[:], lhsT=scr[:, 0:P], rhs=scr[:],
                         start=(i == 0), stop=(i == 7))

    # ---- input DMAs: one queue, strict priority order, chunked ----
    nc.sync.dma_start(out=wq[:, 0:1024], in_=wqT[:, 0:1024])
    nc.sync.dma_start(out=wk[:, 0:1024], in_=wkT[:, 0:1024])
    for c in range(NET):
        nc.sync.dma_start(out=xq[:, c * 1024:(c + 1) * 1024],
                          in_=xqT[:, c * 1024:(c + 1) * 1024])
    for c in range(NET):
        nc.sync.dma_start(out=xk[:, c * 1024:(c + 1) * 1024],
                          in_=xkT[:, c * 1024:(c + 1) * 1024])
    nc.sync.dma_start(out=wq[:, 1024:4096], in_=wqT[:, 1024:4096])
    nc.sync.dma_start(out=wk[:, 1024:4096], in_=wkT[:, 1024:4096])
    nc.sync.dma_start(out=wv[:, 0:2048], in_=wvT[:, 0:2048])
    nc.sync.dma_start(out=wv[:, 2048:4096], in_=wvT[:, 2048:4096])
    for c in range(NET):
        nc.sync.dma_start(out=xv[:, c * 1024:(c + 1) * 1024],
                          in_=xvT[:, c * 1024:(c + 1) * 1024])
    nc.sync.dma_start(out=wo_t[:, 0:2048], in_=woT[:, 0:2048])
    nc.sync.dma_start(out=wo_t[:, 2048:4096], in_=woT[:, 2048:4096])

    # ---- Q/K projections (pair-major weights: block (p, et) at
    # w[:, p*1024 + et*128]) ----
    def proj_pair(w, x, dst, p):
        for sh in range(2):
            ps = pp_mm.tile([P, 512], F32, name="ps", tag="mm")
            for et in range(NET):
                nc.tensor.matmul(
                    ps[:],
                    lhsT=w[:, p * 1024 + et * P:p * 1024 + (et + 1) * P],
                    rhs=x[:, et * 1024 + sh * 512:et * 1024 + (sh + 1) * 512],
                    start=(et == 0), stop=(et == NET - 1),
                )
            nc.vector.tensor_copy(
                dst[:, p * 1024 + sh * 512:p * 1024 + (sh + 1) * 512], ps[:])

    # ---- V projection: natural [t, hd] layout into vaug blocks ----
    def vproj_tt(tt):
        ps = pp_mm.tile([P, 512], F32, name="psv", tag="mm")
        for et in range(NET):
            nc.tensor.matmul(
                ps[:],
                lhsT=xv[:, et * 1024 + tt * P:et * 1024 + (tt + 1) * P],
                rhs=wv[:, et * 512:(et + 1) * 512],
                start=(et == 0), stop=(et == NET - 1),
            )
        # psum cols h*64+d ; even heads -> block cols 0:64, odd -> 64:128
        dstt = vaug[:, tt * 1024:(tt + 1) * 1024].rearrange(
            "p (j q c) -> p j q c", q=2, c=P)
        srcv = ps[:].rearrange("p (j q c) -> p j q c", q=2, c=HD)
        nc.vector.tensor_copy(dstt[:, :, 0, 0:HD], srcv[:, :, 0, :])
        nc.vector.tensor_copy(dstt[:, :, 1, HD:P], srcv[:, :, 1, :])

    # ---- attention pieces ----
    def scores_tt(sh, p, tt):
        qcol = p * 1024 + sh * 512
        kcol = p * 1024 + tt * P
        sAB = pp_sc.tile([P, 1024], F32, name="sAB", tag="sc")
        nc.tensor.matmul(sAB[:, 0:512],
                         lhsT=kt[0:HD, kcol:kcol + P],
                         rhs=qt[0:HD, qcol:qcol + 512],
                         start=True, stop=True)
        nc.tensor.matmul(sAB[:, 512:1024],
                         lhsT=kt[HD:P, kcol:kcol + P],
                         rhs=qt[HD:P, qcol:qcol + 512],
                         start=True, stop=True)
        eAB = etp.tile([P, 1024], BF16, name="eAB", tag="et")
        nc.scalar.activation(eAB[:], sAB[:], EXP, scale=SCALE)
        return eAB

    def ctx_mms(p, tt, eAB, ctxA, ctxB):
        bA = (tt * HPC + 2 * p) * P
        bB = bA + P
        nc.tensor.matmul(ctxA[:], lhsT=vaug[:, bA:bA + P],
                         rhs=eAB[:, 0:512],
                         start=(tt == 0), stop=(tt == NTT - 1))
        nc.tensor.matmul(ctxB[:], lhsT=vaug[:, bB:bB + P],
                         rhs=eAB[:, 512:1024],
                         start=(tt == 0), stop=(tt == NTT - 1))

    def normalize_a(ctx_ps, qcol):
        # ctx rows 0:64, denominator rows 64:128. reciprocal_approx_fast
        # only works at base partition 0: move the denominator down first
        # (cross-partition broadcast rides the gpsimd DMA queue).
        rA = rcp.tile([P, 512], F32, name="rA", tag="rc")
        rA2 = rcp.tile([P, 512], F32, name="rA2", tag="rc")
        nc.vector.tensor_copy(rA[HD:P, :], ctx_ps[HD:P, :])
        nc.gpsimd.dma_start(out=rA[0:HD, :], in_=rA[HD:P, :])
        nc.vector.reciprocal_approx_fast(rA2[0:HD, :], rA[0:HD, :])
        nc.vector.tensor_mul(cat[0:HD, qcol:qcol + 512],
                             ctx_ps[0:HD, :], rA2[0:HD, :])

    def normalize_b(ctx_ps, qcol):
        # mirrored: denominator rows 0:64, ctx rows 64:128
        rB = rcp.tile([P, 512], F32, name="rB", tag="rc")
        nc.vector.reciprocal_approx_fast(rB[0:HD, :], ctx_ps[0:HD, :])
        nc.gpsimd.dma_start(out=rB[HD:P, :], in_=rB[0:HD, :])
        nc.vector.tensor_mul(cat[HD:P, qcol:qcol + 512],
                             ctx_ps[HD:P, :], rB[HD:P, :])

    def ctx_block(sh, p, eabs):
        qcol = p * 1024 + sh * 512
        ctxA = pp_ctx.tile([P, 512], F32, name="ctxA", tag="ctx")
        ctxB = pp_ctx.tile([P, 512], F32, name="ctxB", tag="ctx")
        for tt in range(NTT):
            ctx_mms(p, tt, eabs[tt], ctxA, ctxB)
        normalize_a(ctxA, qcol)
        normalize_b(ctxB, qcol)

    def attn_inline(sh, p):
        qcol = p * 1024 + sh * 512
        ctxA = pp_ctx.tile([P, 512], F32, name="ctxA", tag="ctx")
        ctxB = pp_ctx.tile([P, 512], F32, name="ctxB", tag="ctx")
        for tt in range(NTT):
            eAB = scores_tt(sh, p, tt)
            ctx_mms(p, tt, eAB, ctxA, ctxB)
        normalize_a(ctxA, qcol)
        normalize_b(ctxB, qcol)

    # ---- output projection ----
    def wave0_st(st):
        # out rows st*128:(st+1)*128 (st 0..3), one-bank groups per ih
        # half; slots under the sh=1 attention blocks on pp_mm.
        for ih in range(2):
            ps = pp_mm.tile([P, 512], F32, name="po", tag="mm")
            for p4 in range(4):
                nc.tensor.matmul(
                    ps[:],
                    lhsT=cat[:, p4 * 1024 + st * P:p4 * 1024 + (st + 1) * P],
                    rhs=wo_t[:, p4 * 1024 + ih * 512:p4 * 1024 + (ih + 1) * 512],
                    start=(p4 == 0), stop=(p4 == 3))
            ob = obp.tile([P, 512], BF16, name="ob", tag="ob")
            nc.vector.tensor_copy(ob[:], ps[:])
            nc.sync.dma_start(
                out=out[st * P:(st + 1) * P, ih * 512:(ih + 1) * 512],
                in_=ob[:])

    def wave1(sts):
        # out rows 512:1023 on the freed score banks ([128,1024] groups).
        # Pair 3 (whose sh=1 normalize is the last thing the attention
        # produces) is deferred for the first two groups so their other
        # 12 matmuls overlap the final normalize.
        tiles = {}
        for st in sts[:2]:
            ps = pp_sc.tile([P, 1024], F32, name="po2", tag="sc")
            tiles[st] = ps
            for p4 in range(3):
                for ih in range(2):
                    nc.tensor.matmul(
                        ps[:, ih * 512:(ih + 1) * 512],
                        lhsT=cat[:, p4 * 1024 + st * P:p4 * 1024 + (st + 1) * P],
                        rhs=wo_t[:, p4 * 1024 + ih * 512:p4 * 1024 + (ih + 1) * 512],
                        start=(p4 == 0), stop=False)
        for st in sts[:2]:
            ps = tiles[st]
            for ih in range(2):
                nc.tensor.matmul(
                    ps[:, ih * 512:(ih + 1) * 512],
                    lhsT=cat[:, 3 * 1024 + st * P:3 * 1024 + (st + 1) * P],
                    rhs=wo_t[:, 3 * 1024 + ih * 512:3 * 1024 + (ih + 1) * 512],
                    start=False, stop=True)
            ob = obp.tile([P, 1024], BF16, name="ob2", tag="ob2")
            nc.vector.tensor_copy(ob[:], ps[:])
            nc.sync.dma_start(out=out[st * P:(st + 1) * P, :], in_=ob[:])
        for st in sts[2:]:
            ps = pp_sc.tile([P, 1024], F32, name="po3", tag="sc")
            for p4 in range(4):
                for ih in range(2):
                    nc.tensor.matmul(
                        ps[:, ih * 512:(ih + 1) * 512],
                        lhsT=cat[:, p4 * 1024 + st * P:p4 * 1024 + (st + 1) * P],
                        rhs=wo_t[:, p4 * 1024 + ih * 512:p4 * 1024 + (ih + 1) * 512],
                        start=(p4 == 0), stop=(p4 == 3))
            ob = obp.tile([P, 1024], BF16, name="ob3", tag="ob2")
            nc.vector.tensor_copy(ob[:], ps[:])
            nc.sync.dma_start(out=out[st * P:(st + 1) * P, :], in_=ob[:])

    # ---- the pipeline ----
    proj_pair(wq, xq, qt, 0)
    proj_pair(wk, xk, kt, 0)
    eabs00 = [scores_tt(0, 0, tt) for tt in range(NTT)]
    proj_pair(wq, xq, qt, 1)
    proj_pair(wk, xk, kt, 1)
    eabs01 = [scores_tt(0, 1, tt) for tt in range(NTT)]
    for tt in range(NTT):
        vproj_tt(tt)
    ctx_block(0, 0, eabs00)
    ctx_block(0, 1, eabs01)
    proj_pair(wq, xq, qt, 2)
    proj_pair(wk, xk, kt, 2)
    attn_inline(0, 2)
    proj_pair(wq, xq, qt, 3)
    proj_pair(wk, xk, kt, 3)
    attn_inline(0, 3)
    attn_inline(1, 0)
    wave0_st(0)
    attn_inline(1, 1)
    wave0_st(1)
    attn_inline(1, 2)
    wave0_st(2)
    attn_inline(1, 3)
    wave0_st(3)
    wave1([4, 5, 6, 7])


_CACHE = {}


def build():
    if "nc" in _CACHE:
        return _CACHE["nc"]
    nc = bacc.Bacc("TRN2", target_bir_lowering=False, debug=False)
    xqT = nc.dram_tensor("xqT", [P, NET * S], BF16, kind="ExternalInput").ap()
    xkT = nc.dram_tensor("xkT", [P, NET * S], BF16, kind="ExternalInput").ap()
    xvT = nc.dram_tensor("xvT", [P, NET * S], BF16, kind="ExternalInput").ap()
    wqT = nc.dram_tensor("wqT", [P, NET * HPC * HD], BF16, kind="ExternalInput").ap()
    wkT = nc.dram_tensor("wkT", [P, NET * HPC * HD], BF16, kind="ExternalInput").ap()
    wvT = nc.dram_tensor("wvT", [P, NET * HPC * HD], BF16, kind="ExternalInput").ap()
    woT = nc.dram_tensor("woT", [P, 4 * E], BF16, kind="ExternalInput").ap()
    out = nc.dram_tensor("out", [S, E], BF16, kind="ExternalOutput").ap()
    with tile.TileContext(nc) as tc, ExitStack() as ctx:
        _emit(nc, tc, ctx, (xqT, xkT, xvT, wqT, wkT, wvT, woT, out))
    nc.compile()
    _CACHE["nc"] = nc
    return nc


def make_in_maps(query, key, value, Wq, Wk, Wv, Wo):
    in_maps = []
    for c in range(8):
        b, g = divmod(c, 2)
        hs = slice(g * HPC, (g + 1) * HPC)

        def bf(a):
            return np.ascontiguousarray(a).astype(BF)

        def sbuf_tile(a):
            # [E_or_512, N] -> the SBUF-resident layout [128, n_et * N]:
            # row p, col et*N+c  =  a[et*128 + p, c]
            et = a.shape[0] // P
            return bf(a.reshape(et, P, -1).transpose(1, 0, 2).reshape(P, -1))

        def w_pairmajor(W):
            # [8, 64, E] -> [128, p*1024 + et*128 + (h_in_pair*64 + d)]
            W8 = np.asarray(W, np.float32)
            blocks = []
            for p in range(NPAIR):
                a = W8[2 * p:2 * p + 2].transpose(2, 0, 1).reshape(E, 2 * HD)
                blocks.append(a.reshape(NET, P, 2 * HD).transpose(1, 0, 2)
                              .reshape(P, NET * 2 * HD))
            return bf(np.concatenate(blocks, axis=1))

        # x^T [E, S]; wq/wk pair-major; wv et-major [E, 512] with
        # col h*64+d; woT [512, E] with woT[hd, i] = Wo[i, g*512+hd]
        in_maps.append({
            "xqT": sbuf_tile(np.asarray(query[b], np.float32).T),
            "xkT": sbuf_tile(np.asarray(key[b], np.float32).T),
            "xvT": sbuf_tile(np.asarray(value[b], np.float32).T),
            "wqT": w_pairmajor(np.asarray(Wq[hs], np.float32)),
            "wkT": w_pairmajor(np.asarray(Wk[hs], np.float32)),
            "wvT": sbuf_tile(np.asarray(Wv[hs], np.float32).transpose(2, 0, 1).reshape(E, HPC * HD)),
            "woT": sbuf_tile(np.asarray(Wo[:, g * HPC * HD:(g + 1) * HPC * HD], np.float32).T),
        })
    return in_maps


def kernel(query, key, value, Wq, Wk, Wv, Wo):
    nc = build()
    in_maps = make_in_maps(query, key, value, Wq, Wk, Wv, Wo)
    res = run_bass_kernel_spmd(nc, in_maps, list(range(8))).results
    out = np.empty((B, S, E), np.float32)
    for b in range(B):
        out[b] = (res[2 * b]["out"].astype(np.float32)
                  + res[2 * b + 1]["out"].astype(np.float32))
    return out
